# revision 5
# baseline (speedup 1.0000x reference)
"""Trainium2 Bass kernel for CompoundClassifier GNN message passing, v2.

Model: out = sigmoid(relu(concat(x_ing[src], x_cmp[dst]) @ W1 + b1) @ W2 + b2)

Key facts driving the design (measured):
- Any per-row DMA gather (SWDGE dma_gather) costs ~277ns/row regardless of
  index locality -> 68ms for 2M rows. The descriptor path is the bottleneck,
  not HBM.
- So v2 eliminates index-driven DMA entirely: the gather becomes PE matmuls
  with one-hot selector matrices (gather-as-matmul), with all table blocks
  RESIDENT in SBUF at compile-time addresses.

Algorithm:
- Precompute per-node projections A_ing = x_ing @ W1[:H],
  A_cmp = x_cmp @ W1[H:] + b1 (once per node, on host, fp16).
- Bucket edges by (src_block, dst_block) of 128 nodes. Process buckets in
  raster order (a_local, b) so both stationary operands are compile-time
  SBUF slices: u[128, slots] = A_cmp_blk[b].T-onehot-matmul + A_ing_blk[a]
  .T-onehot-matmul accumulated in PSUM (out[h, slot], hidden on partitions).
- Each bucket gets one 128-slot "quarter"; 4 quarters = one 512-col PSUM
  tile. relu on DVE (PSUM->SBUF fp16), then logit = w2.T @ relu_u on PE
  ([1, 512]), sigmoid(+b2) on ACT, batched DMA out.
- Bucket overflow (>128 edges in a bucket) goes to a small leftover pass
  with streamed stationaries (capacity LQ*128 edges; ~0 used for random
  data, capacity auto-sized from the data).
- Host does index bookkeeping only (permutation + one-hot bitmaps); all
  per-edge compute is on device.

Sharding: src blocks striped across 8 cores (a_global = core + 8*i);
A_cmp replicated; per-core identical program (SPMD).
"""

import sys

for _p in ("/opt/trn_rl_repo",):
    if _p not in sys.path:
        sys.path.insert(0, _p)

import numpy as np

import concourse.bacc as bacc
import concourse.mybir as mybir
import concourse.tile as tile
from concourse.bass_utils import run_bass_kernel_spmd

H = 128
N_ING = 20000
N_CMP = 10000
N_EDGE = 1000000
NCORES = 8

NBA = 160          # padded src blocks (157 real), 20 per core
NBA_CORE = NBA // NCORES       # 20
NBB = 80           # padded dst blocks (79 real)
QPT = 8            # quarters per tile
TPB = NBB // QPT   # 10 tiles per a_local
T = NBA_CORE * TPB             # 200 tiles per core
TILE_N = QPT * 128             # 1024 slots per tile
HALF = TILE_N // 2
OUT_BATCH = 10     # tiles per output DMA

f32 = mybir.dt.float32
f16 = mybir.dt.float16
f8 = mybir.dt.float8e4
AF = mybir.ActivationFunctionType

_prog_cache = {}
_last_in_maps = None


def _build_program(lq):
    assert T % OUT_BATCH == 0
    nc = bacc.Bacc("TRN2", target_bir_lowering=False, debug=False)
    a_ing_d = nc.dram_tensor("a_ing_sl", [128, NBA_CORE * 128], f16, kind="ExternalInput")
    a_cmp_d = nc.dram_tensor("a_cmp", [128, NBB * 128], f16, kind="ExternalInput")
    ohs_d = nc.dram_tensor("ohs", [T, 128, 2 * TILE_N], f8, kind="ExternalInput")
    w2_d = nc.dram_tensor("w2t", [128, 1], f16, kind="ExternalInput")
    b2_d = nc.dram_tensor("b2t", [1, 1], f32, kind="ExternalInput")
    lo_sstat_d = nc.dram_tensor("lo_sstat", [lq, 128, 128], f16, kind="ExternalInput")
    lo_dstat_d = nc.dram_tensor("lo_dstat", [lq, 128, 128], f16, kind="ExternalInput")
    lo_soh_d = nc.dram_tensor("lo_soh", [lq, 128, 128], f8, kind="ExternalInput")
    lo_doh_d = nc.dram_tensor("lo_doh", [lq, 128, 128], f8, kind="ExternalInput")
    outd = nc.dram_tensor("out", [1, T * TILE_N], f32, kind="ExternalOutput")
    lo_outd = nc.dram_tensor("lo_out", [1, lq * 128], f32, kind="ExternalOutput")

    with tile.TileContext(nc) as tc:
        with (
            tc.tile_pool(name="const", bufs=1) as constp,
            tc.tile_pool(name="oh", bufs=4) as ohp,
            tc.tile_pool(name="relu", bufs=4) as relup,
            tc.tile_pool(name="row", bufs=2) as rowp,
            tc.tile_pool(name="lost", bufs=4) as lostp,
            tc.tile_pool(name="upsum", bufs=3, space="PSUM") as upsump,
            tc.tile_pool(name="lpsum", bufs=2, space="PSUM") as lpsump,
        ):
            a_ing = constp.tile([128, NBA_CORE * 128], f16)
            a_cmp = constp.tile([128, NBB * 128], f16)
            # First tiles need a_cmp blocks 0..QPT and a_ing block 0: load
            # those first on the sync queue; stream the bulk on the Pool
            # (SWDGE) queue so it doesn't block per-tile one-hot DMAs.
            nc.sync.dma_start(out=a_cmp[:, : QPT * 128], in_=a_cmp_d[:, : QPT * 128])
            nc.sync.dma_start(out=a_ing[:, :128], in_=a_ing_d[:, :128])
            nc.gpsimd.dma_start(
                out=a_cmp[:, QPT * 128 :], in_=a_cmp_d[:, QPT * 128 :]
            )
            nc.gpsimd.dma_start(out=a_ing[:, 128:], in_=a_ing_d[:, 128:])
            w2t = constp.tile([128, 1], f16)
            nc.sync.dma_start(out=w2t[:], in_=w2_d[:])
            b2t = constp.tile([1, 1], f32)
            nc.sync.dma_start(out=b2t[:], in_=b2_d[:])

            for tb in range(T // OUT_BATCH):
                rowbuf = rowp.tile([1, OUT_BATCH * TILE_N], f32, tag="rowbuf")
                for ti in range(OUT_BATCH):
                    t = tb * OUT_BATCH + ti
                    al = t // TPB
                    b0 = (t % TPB) * QPT

                    ohs = ohp.tile([128, 2 * TILE_N], f8, tag="ohs")
                    nc.sync.dma_start(out=ohs[:], in_=ohs_d[t, :, :])
                    soh = ohs[:, :TILE_N]
                    doh = ohs[:, TILE_N:]

                    u = upsump.tile([128, TILE_N], f32, tag="u")
                    for h in range(2):
                        nc.tensor.matmul(
                            out=u[:, h * HALF : (h + 1) * HALF],
                            lhsT=a_ing[:, al * 128 : (al + 1) * 128],
                            rhs=soh[:, h * HALF : (h + 1) * HALF],
                            start=True,
                            stop=False,
                        )
                    for j in range(QPT):
                        nc.tensor.matmul(
                            out=u[:, j * 128 : (j + 1) * 128],
                            lhsT=a_cmp[:, (b0 + j) * 128 : (b0 + j + 1) * 128],
                            rhs=doh[:, j * 128 : (j + 1) * 128],
                            start=False,
                            stop=(j == 3 or j == QPT - 1),
                        )

                    ru = relup.tile([128, TILE_N], f16, tag="ru")
                    nc.vector.tensor_scalar_max(out=ru[:], in0=u[:], scalar1=0.0)

                    for h in range(2):
                        lg = lpsump.tile([1, HALF], f32, tag="lg")
                        nc.tensor.matmul(
                            out=lg[:],
                            lhsT=w2t[:],
                            rhs=ru[:, h * HALF : (h + 1) * HALF],
                            start=True,
                            stop=True,
                        )
                        nc.scalar.activation(
                            rowbuf[:, ti * TILE_N + h * HALF : ti * TILE_N + (h + 1) * HALF],
                            lg[:],
                            AF.Sigmoid,
                            bias=b2t[:, 0:1],
                        )
                nc.sync.dma_start(
                    out=outd[:, tb * OUT_BATCH * TILE_N : (tb + 1) * OUT_BATCH * TILE_N],
                    in_=rowbuf[:],
                )

            # Leftover pass: streamed stationaries for bucket overflow.
            lrow = rowp.tile([1, lq * 128], f32, tag="lrow")
            for q in range(lq):
                sstat = lostp.tile([128, 128], f16, tag="sstat")
                nc.sync.dma_start(out=sstat[:], in_=lo_sstat_d[q, :, :])
                dstat = lostp.tile([128, 128], f16, tag="dstat")
                nc.sync.dma_start(out=dstat[:], in_=lo_dstat_d[q, :, :])
                soh = lostp.tile([128, 128], f8, tag="lsoh")
                nc.sync.dma_start(out=soh[:], in_=lo_soh_d[q, :, :])
                doh = lostp.tile([128, 128], f8, tag="ldoh")
                nc.sync.dma_start(out=doh[:], in_=lo_doh_d[q, :, :])

                u = upsump.tile([128, TILE_N], f32, tag="u")
                nc.tensor.matmul(
                    out=u[:, :128], lhsT=sstat[:], rhs=soh[:], start=True, stop=False
                )
                nc.tensor.matmul(
                    out=u[:, :128], lhsT=dstat[:], rhs=doh[:], start=False, stop=True
                )
                ru = relup.tile([128, TILE_N], f16, tag="ru")
                nc.vector.tensor_scalar_max(out=ru[:, :128], in0=u[:, :128], scalar1=0.0)
                lg = lpsump.tile([1, HALF], f32, tag="lg")
                nc.tensor.matmul(
                    out=lg[:, :128], lhsT=w2t[:], rhs=ru[:, :128], start=True, stop=True
                )
                nc.scalar.activation(
                    lrow[:, q * 128 : (q + 1) * 128],
                    lg[:, :128],
                    AF.Sigmoid,
                    bias=b2t[:, 0:1],
                )
            nc.sync.dma_start(out=lo_outd[:], in_=lrow[:])

    nc.compile()
    return nc


def _pack_table_blocks(tab16, blocks):
    """[nb*128, 128] fp16 -> [128, nb*128] where partition k, col-block i
    holds tab16[128*blocks[i] + k, :]."""
    nb = len(blocks)
    out = np.empty((128, nb * 128), dtype=np.float16)
    for i, bl in enumerate(blocks):
        out[:, i * 128 : (i + 1) * 128] = tab16[128 * bl : 128 * (bl + 1), :]
    return out


def kernel(x_ingredient, x_compound, edge_index, W1, b1, W2, b2):
    global _last_in_maps
    x_ing = np.asarray(x_ingredient, dtype=np.float32)
    x_cmp = np.asarray(x_compound, dtype=np.float32)
    W1 = np.asarray(W1, dtype=np.float32)
    b1 = np.asarray(b1, dtype=np.float32)
    W2 = np.asarray(W2, dtype=np.float32).reshape(H)
    b2 = np.asarray(b2, dtype=np.float32)
    src = np.asarray(edge_index[0]).astype(np.int64)
    dst = np.asarray(edge_index[1]).astype(np.int64)

    # Per-node projections (once per node instead of once per edge).
    a_ing = x_ing @ W1[:H]
    a_cmp = x_cmp @ W1[H:] + b1

    a_ing16 = np.zeros((NBA * 128, H), dtype=np.float16)
    a_ing16[:N_ING] = a_ing.astype(np.float16)
    a_cmp16 = np.zeros((NBB * 128, H), dtype=np.float16)
    a_cmp16[:N_CMP] = a_cmp.astype(np.float16)

    # ---- bucket bookkeeping (host) ----
    a_g = src >> 7                      # 0..156
    bb = dst >> 7                       # 0..78
    core = (a_g & 7).astype(np.int64)   # a_global = core + 8*a_local
    a_local = a_g >> 3
    qi = a_local * NBB + bb             # quarter index within core
    tt = qi // QPT
    jj = qi % QPT

    # rank of each edge within its bucket
    bucket = a_g * NBB + bb
    order = np.argsort(bucket, kind="stable")
    sb = bucket[order]
    starts = np.concatenate(([0], np.nonzero(np.diff(sb))[0] + 1))
    counts = np.diff(np.concatenate((starts, [N_EDGE])))
    rank = np.empty(N_EDGE, dtype=np.int64)
    rank[order] = np.arange(N_EDGE) - np.repeat(starts, counts)

    main = rank < 128
    slot = jj * 128 + rank              # valid for main edges

    # leftover quarters, per core
    lo_edges = np.nonzero(~main)[0]
    lo_needed = np.zeros(NCORES, dtype=np.int64)
    lo_q = np.zeros(N_EDGE, dtype=np.int64)
    lo_r = np.zeros(N_EDGE, dtype=np.int64)
    if lo_edges.size:
        # group leftover edges by (bucket, chunk)
        ch = (rank[lo_edges] - 128) >> 7
        key = bucket[lo_edges] * 64 + ch
        okey = np.argsort(key, kind="stable")
        le = lo_edges[okey]
        ku = key[okey]
        # assign quarter ids per core in order of appearance
        qid = np.empty(le.size, dtype=np.int64)
        per_core_ctr = {}
        cur_key, cur_q = None, -1
        for i in range(le.size):
            c = int(core[le[i]])
            if ku[i] != cur_key:
                cur_key = ku[i]
                cur_q = per_core_ctr.get(c, 0)
                per_core_ctr[c] = cur_q + 1
            qid[i] = cur_q
        lo_q[le] = qid
        lo_r[le] = (rank[le] - 128) & 127
        lo_needed = np.zeros(NCORES, dtype=np.int64)
        for c in range(NCORES):
            lo_needed[c] = per_core_ctr.get(c, 0)
    LQ = max(8, int(-(-int(lo_needed.max()) // 8) * 8))

    # ---- one-hot bitmaps (uint8 fp8e4m3 bit patterns; 1.0 = 0x38) ----
    ONE = np.uint8(0x38)
    ohs = np.zeros((NCORES, T, 128, 2 * TILE_N), dtype=np.uint8)
    me = np.nonzero(main)[0]
    flat_s = ((core[me] * T + tt[me]) * 128 + (src[me] & 127)) * (2 * TILE_N) + slot[me]
    ohs.reshape(-1)[flat_s] = ONE
    flat_d = ((core[me] * T + tt[me]) * 128 + (dst[me] & 127)) * (2 * TILE_N) + TILE_N + slot[me]
    ohs.reshape(-1)[flat_d] = ONE

    lo_soh = np.zeros((NCORES, LQ, 128, 128), dtype=np.uint8)
    lo_doh = np.zeros((NCORES, LQ, 128, 128), dtype=np.uint8)
    lo_sstat = np.zeros((NCORES, LQ, 128, 128), dtype=np.float16)
    lo_dstat = np.zeros((NCORES, LQ, 128, 128), dtype=np.float16)
    if lo_edges.size:
        fl_s = ((core[lo_edges] * LQ + lo_q[lo_edges]) * 128 + (src[lo_edges] & 127)) * 128 + lo_r[lo_edges]
        lo_soh.reshape(-1)[fl_s] = ONE
        fl_d = ((core[lo_edges] * LQ + lo_q[lo_edges]) * 128 + (dst[lo_edges] & 127)) * 128 + lo_r[lo_edges]
        lo_doh.reshape(-1)[fl_d] = ONE
        for i in lo_edges:
            c, q = int(core[i]), int(lo_q[i])
            lo_sstat[c, q] = a_ing16[128 * a_g[i] : 128 * (a_g[i] + 1), :]
            lo_dstat[c, q] = a_cmp16[128 * bb[i] : 128 * (bb[i] + 1), :]

    w2t = W2.astype(np.float16).reshape(128, 1)
    b2t = np.full((1, 1), float(b2.reshape(-1)[0]), dtype=np.float32)
    a_cmp_packed = _pack_table_blocks(a_cmp16, list(range(NBB)))

    in_maps = []
    for c in range(NCORES):
        in_maps.append(
            {
                "a_ing_sl": _pack_table_blocks(a_ing16, [c + 8 * i for i in range(NBA_CORE)]),
                "a_cmp": a_cmp_packed,
                "ohs": ohs[c].view(mybir.dt.np(f8)),
                "w2t": w2t,
                "b2t": b2t,
                "lo_sstat": lo_sstat[c],
                "lo_dstat": lo_dstat[c],
                "lo_soh": lo_soh[c].view(mybir.dt.np(f8)),
                "lo_doh": lo_doh[c].view(mybir.dt.np(f8)),
            }
        )
    _last_in_maps = in_maps

    if LQ not in _prog_cache:
        _prog_cache[LQ] = _build_program(LQ)
    nc = _prog_cache[LQ]
    _prog_cache["prog"] = nc

    res = run_bass_kernel_spmd(nc, in_maps, list(range(NCORES)))

    # ---- unshard ----
    out_main = np.stack([res.results[c]["out"].reshape(-1) for c in range(NCORES)])
    out_lo = np.stack([res.results[c]["lo_out"].reshape(-1) for c in range(NCORES)])
    result = np.empty(N_EDGE, dtype=np.float32)
    result[me] = out_main[core[me], tt[me] * TILE_N + slot[me]]
    if lo_edges.size:
        result[lo_edges] = out_lo[core[lo_edges], lo_q[lo_edges] * 128 + lo_r[lo_edges]]
    return result.reshape(N_EDGE, 1)
